# revision 17
# baseline (speedup 1.0000x reference)
"""ContextualAttention2D Trainium2 kernel.

Full inputs -> full output; internally data-parallel over batch across 8
NeuronCores (2 batches per core), single SPMD NEFF, no collectives.

Math (per batch):
  hidden[n,c]   = x.reshape(C, H*W).T
  hn            = layernorm_c(hidden) * ln_w + ln_b
  q             = hn @ Wq.T ;  k = ctx @ Wk.T ; v = ctx @ Wv.T
  ctx           = context @ Wctx.T      (folded: k = context @ (Wk@Wctx).T etc)
  attn          = softmax_l(q @ k.T * hd^-0.5 + maskbias) ; out = attn @ v
  y             = (out @ Wo.T + hidden).T.reshape(C, H, W)

fp8 (e4m3) DoubleRow matmuls carry the projection GEMMs and attn@V at
0.5 cycles/row with two 128-deep k-tiles per instruction.  Per-tensor
power-of-two scales keep every fp8 operand in range:
  wq8 = 64*Wq*ln_w/8          q_psum = 64*q    rbc = rstd/1024 (Rsqrt scale)
  wck8 = 16*(Wk@Wctx).T       k_sb  = 16*k     (1/16 cancelled by rbc)
  wcv8 = 16*(Wv@Wctx).T       v8    = 16*v     aug ones col = 1/8
  probs = exp(scores+mb) e4m3 den8 = den/8     rcb2 = 8/den  -> an8 = 128*attn
  wo8 = 32*Wo.T               out_psum = 4096*out, residual add scales 1/4096

LayerNorm: per-token mean/var via ones-matmuls (cross-partition sum);
rstd/1024 from ACT Rsqrt (scale=2^20), invr via DVE (var+eps)*rstd; the
mean correction enters Q as a rank-2 bf16 matmul into the same PSUM group.

Softmax denominator: attn@V is augmented with a 1/8 ones column; the
denominator row is DMA-gathered into a [128,32] tile so one DVE
reciprocal covers all 8 heads, then DMA-broadcast back per head-pair.
attn@V PSUM is evicted by DMA (off-engine) and normalized straight into
the fp8 out-projection operand.
"""
import numpy as np
import ml_dtypes

from concourse import bacc, mybir, tile
from concourse.bass_utils import run_bass_kernel_spmd

BF = ml_dtypes.bfloat16
F8 = ml_dtypes.float8_e4m3

B, C, H, W = 16, 512, 32, 32
NH, HD = 8, 64
CTX_DIM, L = 768, 512
EPS = 1e-5
N = H * W                 # 1024 tokens
NCORES = 8
BPC = B // NCORES         # batches per core
P = 128
CC = C // P               # 4 c-chunks
DC = CTX_DIM // P         # 6 d-chunks
LC = L // P               # 4 l-chunks
MC = N // 512             # 2 token chunks of 512
MASK_NEG = -30000.0

F32 = mybir.dt.float32
BF16 = mybir.dt.bfloat16
FP8 = mybir.dt.float8e4
DR = mybir.MatmulPerfMode.DoubleRow

_NC_CACHE = None


def _build():
    nc = bacc.Bacc(None, target_bir_lowering=False, debug=False)

    x8d = nc.dram_tensor("x8", [BPC, C, N], FP8, kind="ExternalInput")
    xbfd = nc.dram_tensor("xbf", [BPC, C, N], BF16, kind="ExternalInput")
    ctx8d = nc.dram_tensor("ctx8", [BPC, CTX_DIM, L], FP8, kind="ExternalInput")
    mbd = nc.dram_tensor("mb", [BPC, L], F32, kind="ExternalInput")
    wq8d = nc.dram_tensor("wq8", [C, C], FP8, kind="ExternalInput")
    wck8d = nc.dram_tensor("wck8", [CTX_DIM, C], FP8, kind="ExternalInput")
    wcv8d = nc.dram_tensor("wcv8", [CTX_DIM, C], FP8, kind="ExternalInput")
    wo8d = nc.dram_tensor("wo8", [C, C], FP8, kind="ExternalInput")
    qr2d = nc.dram_tensor("q_r2", [2, C], BF16, kind="ExternalInput")
    yd = nc.dram_tensor("y", [BPC, C, N], F32, kind="ExternalOutput")

    with tile.TileContext(nc) as tc:
        with (
            tc.tile_pool(name="wpool", bufs=1) as wpool,
            tc.tile_pool(name="xpool", bufs=2) as xpool,
            tc.tile_pool(name="actpool", bufs=2) as actpool,
            tc.tile_pool(name="ppool", bufs=6) as ppool,
            tc.tile_pool(name="spool", bufs=2) as spool,
            tc.tile_pool(name="psum", bufs=2, space="PSUM") as psum,
            tc.tile_pool(name="psc", bufs=4, space="PSUM") as psc,
            tc.tile_pool(name="paug", bufs=2, space="PSUM") as paug,
            tc.tile_pool(name="dpool", bufs=4, space="DRAM") as dpool,
        ):
            # ---- persistent weights ----
            wq_sb = wpool.tile([P, CC, C], FP8)
            nc.scalar.dma_start(wq_sb[:], wq8d.ap().rearrange("(cc p) e -> p cc e", p=P))
            wck_sb = wpool.tile([P, DC, C], FP8)
            nc.scalar.dma_start(wck_sb[:], wck8d.ap().rearrange("(dc p) e -> p dc e", p=P))
            wcv_sb = wpool.tile([P, DC, C], FP8)
            nc.scalar.dma_start(wcv_sb[:], wcv8d.ap().rearrange("(dc p) e -> p dc e", p=P))
            wo_sb = wpool.tile([P, CC, C], FP8)
            nc.scalar.dma_start(wo_sb[:], wo8d.ap().rearrange("(ec p) c -> p ec c", p=P))
            qr2_sb = wpool.tile([2, C], BF16)
            nc.scalar.dma_start(qr2_sb[:], qr2d.ap())

            ones1_sb = wpool.tile([P, 1], BF16)   # stats lhsT (column sums)
            nc.vector.memset(ones1_sb[:], 1.0)
            onesr_sb = wpool.tile([1, P], BF16)    # bcast-matmul lhsT (rank-1)
            nc.vector.memset(onesr_sb[:], 1.0)
            eps2_sb = wpool.tile([1, 1], F32)      # eps * 2^20 (Rsqrt bias)
            nc.vector.memset(eps2_sb[:], EPS * 1048576.0)

            # Per-batch emission closures; emitted in a software-pipelined
            # order so PE filler (projection chains) sits between the
            # ACT-bound score-exp groups and their attn@v consumers.
            def make_batch(b):
                st = {}

                def loads():
                    # b0 bulk loads ride the sync queue; later batches use the
                    # gpsimd (SWDGE) queue so they don't delay the previous
                    # batch's latency-sensitive normalize DMAs on sync.
                    bulk = nc.sync.dma_start if b == 0 else nc.gpsimd.dma_start
                    st["x8"] = xpool.tile([P, CC, N], FP8, name=f"x8{b}", tag="x8")
                    st["xbf"] = xpool.tile([P, CC, N], BF16, name=f"xbf{b}", tag="xbf")
                    for cc in range(CC):
                        bulk(st["xbf"][:, cc, :],
                             xbfd.ap()[b][cc * P:(cc + 1) * P, :])
                    for cc in range(CC):
                        nc.scalar.dma_start(
                            st["x8"][:, cc, :],
                            x8d.ap()[b][cc * P:(cc + 1) * P, :])
                    st["ctx8"] = xpool.tile([P, DC, L], FP8, name=f"ctx8{b}", tag="ctx8")
                    for dc in range(DC):
                        bulk(st["ctx8"][:, dc, :],
                             ctx8d.ap()[b][dc * P:(dc + 1) * P, :])
                    st["mb"] = spool.tile([P, LC], F32, name=f"mb{b}", tag="mb")
                    nc.sync.dma_start(
                        st["mb"][:], mbd.ap()[b].rearrange("(lc p) -> p lc", p=P))
                    st["xsq"] = xpool.tile([P, CC, N], BF16, name=f"xsq{b}",
                                           tag="xsq", bufs=1)
                    for cc in range(CC):
                        nc.gpsimd.tensor_tensor(
                            st["xsq"][:, cc, :], st["xbf"][:, cc, :],
                            st["xbf"][:, cc, :], op=mybir.AluOpType.mult)
                    st["q"] = actpool.tile([P, CC, MC, 512], BF16, name=f"q{b}", tag="q")
                    st["k"] = actpool.tile([P, CC, L], BF16, name=f"k{b}", tag="k")
                    # v8: [d, lc-pair u, k-tile i, head, 96]; col 64 = 1/8 ones
                    # (denominator), cols 65:96 zero pad (DoubleRow stationary
                    # width must be a multiple of 32)
                    st["v8"] = actpool.tile([P, LC // 2, 2, NH, 96], FP8,
                                            name=f"v8{b}", tag="v8")
                    nc.vector.memset(st["v8"][:, :, :, :, HD + 1:], 0.0)
                    nc.vector.memset(st["v8"][:, :, :, :, HD:HD + 1], 0.125)
                    st["an8"] = actpool.tile([P, CC, MC, 512], FP8,
                                             name=f"an8{b}", tag="an8")
                    st["r2"] = {}
                    st["rbc"] = {}
                    st["den"] = {}
                    st["asb"] = {}
                    st["rcb"] = {}

                def stats(mc):
                    ms = slice(mc * 512, (mc + 1) * 512)
                    st1 = psum.tile([1, 512], F32, name=f"st1{b}{mc}", tag="ps")
                    for cc in range(CC):
                        nc.tensor.matmul(st1[:], ones1_sb[:], st["xbf"][:, cc, ms],
                                         start=(cc == 0), stop=(cc == CC - 1))
                    st2 = psum.tile([1, 512], F32, name=f"st2{b}{mc}", tag="ps")
                    for cc in range(CC):
                        nc.tensor.matmul(st2[:], ones1_sb[:], st["xsq"][:, cc, ms],
                                         start=(cc == 0), stop=(cc == CC - 1))
                    negmu = spool.tile([1, 512], BF16, name=f"negmu{b}{mc}", tag="negmu")
                    nc.vector.tensor_scalar_mul(negmu[:], st1[:], -1.0 / C)
                    musq = spool.tile([1, 512], F32, name=f"musq{b}{mc}", tag="musq")
                    nc.vector.tensor_tensor(musq[:], negmu[:], negmu[:],
                                            op=mybir.AluOpType.mult)
                    var = spool.tile([1, 512], F32, name=f"var{b}{mc}", tag="var")
                    nc.vector.scalar_tensor_tensor(
                        var[:], st2[:], 1.0 / C, musq[:],
                        op0=mybir.AluOpType.mult, op1=mybir.AluOpType.subtract)
                    # invr_k = sqrt((var+eps)*2^20) = 1024*invr  (fp32 for recip)
                    invr_k = spool.tile([1, 512], F32, name=f"invk{b}{mc}", tag="invk")
                    nc.scalar.activation(invr_k[:], var[:],
                                         mybir.ActivationFunctionType.Sqrt,
                                         bias=eps2_sb[:], scale=1048576.0)
                    rstd_f = spool.tile([1, 512], F32, name=f"rstf{b}{mc}", tag="rstf")
                    nc.vector.reciprocal_approx_fast(rstd_f[:], invr_k[:])
                    rstd = spool.tile([1, 512], BF16, name=f"rstd{b}{mc}", tag="rstd")
                    nc.vector.tensor_copy(rstd[:], rstd_f[:])
                    # invr/1024 = (var+eps) * (rstd/1024)
                    invr = spool.tile([1, 512], BF16, name=f"invr{b}{mc}", tag="invr")
                    nc.vector.scalar_tensor_tensor(
                        invr[:], var[:], EPS, rstd[:],
                        op0=mybir.AluOpType.add, op1=mybir.AluOpType.mult)
                    r2 = spool.tile([2, 512], BF16, name=f"r2_{b}{mc}", tag="r2")
                    nc.sync.dma_start(r2[0:1, :], negmu[:])
                    nc.sync.dma_start(r2[1:2, :], invr[:])
                    rbp = paug.tile([P, 512], F32, name=f"rbp{b}{mc}", tag="aug")
                    nc.tensor.matmul(rbp[:], onesr_sb[:], rstd[:],
                                     start=True, stop=True)
                    rbc = spool.tile([P, 512], BF16, name=f"rbc{b}{mc}", tag="rbc")
                    nc.vector.tensor_copy(rbc[:], rbp[:])
                    st["r2"][mc] = r2
                    st["rbc"][mc] = rbc

                def k_chain(ec):
                    es = slice(ec * P, (ec + 1) * P)
                    kp = psum.tile([P, 512], F32, name=f"kp{b}{ec}", tag="ps")
                    for u in range(DC // 2):
                        nc.tensor.matmul(kp[:], wck_sb[:, 2 * u:2 * u + 2, es],
                                         st["ctx8"][:, 2 * u:2 * u + 2, :],
                                         start=(u == 0), stop=(u == DC // 2 - 1),
                                         perf_mode=DR)
                    nc.vector.tensor_copy(st["k"][:, ec, :], kp[:])

                def v_chain(lc):
                    ls = slice(lc * P, (lc + 1) * P)
                    vp = psum.tile([P, 512], F32, name=f"vp{b}{lc}", tag="ps")
                    for u in range(DC // 2):
                        nc.tensor.matmul(vp[:], st["ctx8"][:, 2 * u:2 * u + 2, ls],
                                         wcv_sb[:, 2 * u:2 * u + 2, :],
                                         start=(u == 0), stop=(u == DC // 2 - 1),
                                         perf_mode=DR)
                    with nc.allow_low_precision(reason="fp8 attn values; error damped by residual"):
                        nc.vector.tensor_copy(
                            st["v8"][:, lc // 2, lc % 2, :, 0:HD],
                            vp[:].rearrange("p (h d) -> p h d", d=HD))

                def q_chain(ec, mc):
                    es = slice(ec * P, (ec + 1) * P)
                    ms = slice(mc * 512, (mc + 1) * 512)
                    qp = psum.tile([P, 512], F32, name=f"qp{b}{ec}{mc}", tag="ps")
                    for u in range(CC // 2):
                        nc.tensor.matmul(qp[:], wq_sb[:, 2 * u:2 * u + 2, es],
                                         st["x8"][:, 2 * u:2 * u + 2, ms],
                                         start=(u == 0), stop=False,
                                         perf_mode=DR)
                    nc.tensor.matmul(qp[:], qr2_sb[:, es], st["r2"][mc][:],
                                     start=False, stop=True)
                    nc.vector.tensor_tensor(st["q"][:, ec, mc, :], qp[:],
                                            st["rbc"][mc][:],
                                            op=mybir.AluOpType.mult)

                def sc_exp_group(mc, j):
                    if mc not in st["den"]:
                        st["den"][mc] = spool.tile([P, NH * 4], F32,
                                                   name=f"den{b}{mc}", tag="den")
                        st["asb"][mc] = {}
                    # probs for this head-pair: per hh a [P, 2(u: lc pair), 512]
                    # fp8 tile feeding the DoubleRow attn@v directly.
                    ps_h = []
                    for hh in range(2):
                        h = 2 * j + hh
                        po = hh * HD
                        p8 = ppool.tile([P, 2, 2, 512], FP8,
                                        name=f"p8{b}{mc}{j}{hh}", tag="p8")
                        ts = []
                        for lc in range(LC):
                            t = psc.tile([P, 512], F32,
                                         name=f"sc{b}{mc}{j}{hh}{lc}", tag="sc")
                            nc.tensor.matmul(
                                t[:],
                                st["k"][po:po + HD, j, lc * P:(lc + 1) * P],
                                st["q"][po:po + HD, j, mc, :],
                                start=True, stop=True)
                            ts.append(t)
                        for lc, t in enumerate(ts):
                            nc.scalar.activation(
                                p8[:, lc // 2, lc % 2, :], t[:],
                                mybir.ActivationFunctionType.Exp,
                                bias=st["mb"][:, lc:lc + 1])
                        ps_h.append(p8)
                    return ps_h

                def attnv_group(mc, j, ps_h):
                    for hh in range(2):
                        h = 2 * j + hh
                        aug = paug.tile([96, 512], F32,
                                        name=f"aug{b}{mc}{j}{hh}", tag="aug")
                        for u in range(2):
                            nc.tensor.matmul(aug[:], st["v8"][:, u, :, h, :],
                                             ps_h[hh][:, u, :, :],
                                             start=(u == 0), stop=(u == 1),
                                             perf_mode=DR)
                        asb = ppool.tile([HD + 1, 512], F32,
                                         name=f"asb{b}{mc}{j}{hh}", tag="asb",
                                         bufs=16)
                        nc.vector.tensor_copy(asb[:], aug[0:HD + 1, :])
                        # denominator row -> [128, 4] slice of the den gather
                        nc.sync.dma_start(
                            st["den"][mc][:, h * 4:(h + 1) * 4],
                            asb[HD:HD + 1, :])
                        st["asb"][mc][h] = asb

                def norm(mc):
                    rcp = spool.tile([P, NH * 4], BF16, name=f"rcp{b}{mc}", tag="rcp")
                    with nc.allow_low_precision(reason="softmax denominators; error damped by residual"):
                        nc.vector.reciprocal(rcp[:], st["den"][mc][:])
                    rcp_d = dpool.tile([P, NH, 4], BF16, name=f"rcpd{b}{mc}", tag="rcpd")
                    nc.sync.dma_start(
                        rcp_d[:].rearrange("p h f -> p (h f)"), rcp[:])
                    for h in range(NH):
                        j, hh = h // 2, h % 2
                        rcb = spool.tile([HD, 512], BF16,
                                         name=f"rcb{b}{mc}{h}", tag="rcb",
                                         bufs=6)
                        nc.sync.dma_start(
                            rcb[:].rearrange("d (p f) -> d p f", f=4),
                            rcp_d[:, h, :].rearrange("p (one f) -> one p f", one=1)
                            .to_broadcast((HD, P, 4)))
                        # split normalize between DVE and the idle gpsimd
                        eng = nc.vector if hh == 0 else nc.gpsimd
                        with nc.allow_low_precision(reason="fp8 attn operand; error damped by residual"):
                            eng.tensor_tensor(
                                st["an8"][hh * HD:(hh + 1) * HD, j, mc, :],
                                st["asb"][mc][h][0:HD, :], rcb[:],
                                op=mybir.AluOpType.mult)

                def outproj(cc, mc):
                    ms = slice(mc * 512, (mc + 1) * 512)
                    cs = slice(cc * P, (cc + 1) * P)
                    op = psum.tile([P, 512], F32, name=f"op{b}{cc}{mc}", tag="ps")
                    for u in range(CC // 2):
                        nc.tensor.matmul(op[:], wo_sb[:, 2 * u:2 * u + 2, cs],
                                         st["an8"][:, 2 * u:2 * u + 2, mc, :],
                                         start=(u == 0), stop=(u == CC // 2 - 1),
                                         perf_mode=DR)
                    y_sb = xpool.tile([P, 512], F32, name=f"y{b}{cc}{mc}",
                                      tag="y", bufs=4)
                    nc.vector.scalar_tensor_tensor(
                        y_sb[:], op[:], 1.0 / 4096.0, st["xbf"][:, cc, ms],
                        op0=mybir.AluOpType.mult, op1=mybir.AluOpType.add)
                    nc.sync.dma_start(
                        yd.ap()[b][cc * P:(cc + 1) * P, mc * 512:(mc + 1) * 512],
                        y_sb[:])

                return dict(loads=loads, stats=stats, k_chain=k_chain,
                            v_chain=v_chain, q_chain=q_chain,
                            sc_exp_group=sc_exp_group, attnv_group=attnv_group,
                            norm=norm, outproj=outproj)

            # ---- software-pipelined emission (cross-batch modulo schedule) ----
            # PE fillers sit between ACT-bound score/exp groups and their
            # attn@v consumers; fillers are chosen to be independent of the
            # preceding normalize latency.
            E = [make_batch(b) for b in range(BPC)]

            def attn_pass(eb, mc, fillers, post_first=None):
                # attn@v trails the score/exp groups by one j so its P tiles
                # (ACT exps) are complete; fillers keep PE fed in between.
                prev = None
                for j in range(NH // 2):
                    ps_h = eb["sc_exp_group"](mc, j)
                    fillers[j]()
                    if prev is not None:
                        eb["attnv_group"](mc, j - 1, prev)
                        if j == 1 and post_first is not None:
                            post_first()
                    prev = ps_h
                eb["attnv_group"](mc, NH // 2 - 1, prev)

            def nop():
                pass

            E[0]["loads"]()
            E[0]["stats"](0)
            E[0]["stats"](1)
            for lc in range(LC):
                E[0]["v_chain"](lc)
            E[0]["k_chain"](0)
            E[0]["q_chain"](0, 0)
            E[0]["q_chain"](0, 1)

            def kq(eb, j):
                def f():
                    eb["k_chain"](j)
                    eb["q_chain"](j, 0)
                    eb["q_chain"](j, 1)
                return f

            attn_pass(E[0], 0, [kq(E[0], 1), kq(E[0], 2), kq(E[0], 3),
                                E[1]["loads"]])
            attn_pass(E[0], 1,
                      [lambda: (E[1]["stats"](0), E[1]["stats"](1)),
                       lambda: (E[1]["v_chain"](0), E[1]["v_chain"](1)),
                       lambda: (E[1]["v_chain"](2), E[1]["v_chain"](3)),
                       kq(E[1], 0)],
                      post_first=lambda: E[0]["norm"](0))
            attn_pass(E[1], 0,
                      [lambda: (E[0]["outproj"](0, 0), E[1]["k_chain"](1),
                                E[1]["q_chain"](1, 0), E[1]["q_chain"](1, 1)),
                       lambda: (E[0]["outproj"](1, 0), E[1]["k_chain"](2),
                                E[1]["q_chain"](2, 0), E[1]["q_chain"](2, 1)),
                       lambda: (E[0]["outproj"](2, 0), E[1]["k_chain"](3),
                                E[1]["q_chain"](3, 0), E[1]["q_chain"](3, 1)),
                       lambda: E[0]["outproj"](3, 0)],
                      post_first=lambda: E[0]["norm"](1))
            attn_pass(E[1], 1,
                      [lambda: E[0]["outproj"](0, 1),
                       lambda: E[0]["outproj"](1, 1),
                       lambda: (E[0]["outproj"](2, 1), E[1]["outproj"](0, 0)),
                       lambda: (E[0]["outproj"](3, 1), E[1]["outproj"](1, 0))],
                      post_first=lambda: E[1]["norm"](0))
            E[1]["outproj"](2, 0)
            E[1]["outproj"](3, 0)
            E[1]["norm"](1)
            for cc in range(CC):
                E[1]["outproj"](cc, 1)
    nc.compile()
    return nc


def _get_nc():
    global _NC_CACHE
    if _NC_CACHE is None:
        _NC_CACHE = _build()
    return _NC_CACHE


def kernel(x, context, context_mask, ln_w, ln_b, Wq, Wk, Wv, Wo, Wctx):
    x = np.asarray(x, np.float32)
    context = np.asarray(context, np.float32)
    context_mask = np.asarray(context_mask)
    ln_w = np.asarray(ln_w, np.float32)
    ln_b = np.asarray(ln_b, np.float32)
    Wq = np.asarray(Wq, np.float32)
    Wk = np.asarray(Wk, np.float32)
    Wv = np.asarray(Wv, np.float32)
    Wo = np.asarray(Wo, np.float32)
    Wctx = np.asarray(Wctx, np.float32)

    scale = HD ** -0.5
    wq_f = Wq * (ln_w[None, :] * scale)          # [E, C] ln scale + attn scale
    wq8 = np.ascontiguousarray(wq_f.T * 64.0).astype(F8)
    q_r2 = np.stack([64.0 * wq_f.sum(1),
                     65536.0 * ((Wq * scale) @ ln_b)]).astype(BF)   # [2, E]
    wck8 = np.ascontiguousarray((Wk @ Wctx).T * 16.0).astype(F8)    # [768, 512]
    wcv8 = np.ascontiguousarray((Wv @ Wctx).T * 16.0).astype(F8)
    wo8 = np.ascontiguousarray(Wo.T * 32.0).astype(F8)

    xr = x.reshape(NCORES, BPC, C, N)
    x8 = xr.astype(F8)
    xbf = xr.astype(BF)
    ctx8 = np.ascontiguousarray(
        context.transpose(0, 2, 1)).astype(F8).reshape(NCORES, BPC, CTX_DIM, L)
    mb = ((~context_mask).astype(np.float32) * MASK_NEG).reshape(NCORES, BPC, L)

    in_maps = [
        {"x8": np.ascontiguousarray(x8[c]), "xbf": np.ascontiguousarray(xbf[c]),
         "ctx8": np.ascontiguousarray(ctx8[c]),
         "mb": np.ascontiguousarray(mb[c]), "wq8": wq8, "wck8": wck8,
         "wcv8": wcv8, "wo8": wo8, "q_r2": q_r2}
        for c in range(NCORES)
    ]
    res = run_bass_kernel_spmd(_get_nc(), in_maps, core_ids=list(range(NCORES)))
    y = np.stack([r["y"] for r in res.results])          # [8, 2, C, N]
    return y.reshape(B, C, H, W)


# revision 20
# speedup vs baseline: 1.5944x; 1.5944x over previous
"""ContextualAttention2D Trainium2 kernel.

Full inputs -> full output; internally data-parallel over batch across 8
NeuronCores (2 batches per core), single SPMD NEFF, no collectives.

Math (per batch):
  hidden[n,c]   = x.reshape(C, H*W).T
  hn            = layernorm_c(hidden) * ln_w + ln_b
  q             = hn @ Wq.T ;  k = ctx @ Wk.T ; v = ctx @ Wv.T
  ctx           = context @ Wctx.T      (folded: k = context @ (Wk@Wctx).T etc)
  attn          = softmax_l(q @ k.T * hd^-0.5 + maskbias) ; out = attn @ v
  y             = (out @ Wo.T + hidden).T.reshape(C, H, W)

fp8 (e4m3) DoubleRow matmuls carry the projection GEMMs and attn@V at
0.5 cycles/row with two 128-deep k-tiles per instruction.  Per-tensor
power-of-two scales keep every fp8 operand in range:
  wq8 = 64*Wq*ln_w/8          q_psum = 64*q    rbc = rstd/1024 (Rsqrt scale)
  wck8 = 16*(Wk@Wctx).T       k_sb  = 16*k     (1/16 cancelled by rbc)
  wcv8 = 16*(Wv@Wctx).T       v8    = 16*v     aug ones col = 1/8
  probs = exp(scores+mb) e4m3 den8 = den/8     rcb2 = 8/den  -> an8 = 128*attn
  wo8 = 32*Wo.T               out_psum = 4096*out, residual add scales 1/4096

LayerNorm: per-token mean/var via ones-matmuls (cross-partition sum);
rstd/1024 from ACT Rsqrt (scale=2^20), invr via DVE (var+eps)*rstd; the
mean correction enters Q as a rank-2 bf16 matmul into the same PSUM group.

Softmax denominator: attn@V is augmented with a 1/8 ones column; the
denominator row is DMA-gathered into a [128,32] tile so one DVE
reciprocal covers all 8 heads, then DMA-broadcast back per head-pair.
attn@V PSUM is evicted by DMA (off-engine) and normalized straight into
the fp8 out-projection operand.
"""
import numpy as np
import ml_dtypes

from concourse import bacc, mybir, tile
from concourse.bass_utils import run_bass_kernel_spmd

BF = ml_dtypes.bfloat16
F8 = ml_dtypes.float8_e4m3

B, C, H, W = 16, 512, 32, 32
NH, HD = 8, 64
CTX_DIM, L = 768, 512
EPS = 1e-5
N = H * W                 # 1024 tokens
NCORES = 8
BPC = B // NCORES         # batches per core
P = 128
CC = C // P               # 4 c-chunks
DC = CTX_DIM // P         # 6 d-chunks
LC = L // P               # 4 l-chunks
MC = N // 512             # 2 token chunks of 512
MASK_NEG = -30000.0

F32 = mybir.dt.float32
BF16 = mybir.dt.bfloat16
FP8 = mybir.dt.float8e4
DR = mybir.MatmulPerfMode.DoubleRow

_NC_CACHE = None


def _build():
    nc = bacc.Bacc(None, target_bir_lowering=False, debug=False)

    x8d = nc.dram_tensor("x8", [BPC, C, N], FP8, kind="ExternalInput")
    xbfd = nc.dram_tensor("xbf", [BPC, C, N], BF16, kind="ExternalInput")
    ctx8d = nc.dram_tensor("ctx8", [BPC, CTX_DIM, L], FP8, kind="ExternalInput")
    mbd = nc.dram_tensor("mb", [BPC, L], F32, kind="ExternalInput")
    wq8d = nc.dram_tensor("wq8", [C, C], FP8, kind="ExternalInput")
    wck8d = nc.dram_tensor("wck8", [CTX_DIM, C], FP8, kind="ExternalInput")
    wcv8d = nc.dram_tensor("wcv8", [CTX_DIM, C], FP8, kind="ExternalInput")
    wo8d = nc.dram_tensor("wo8", [C, C], FP8, kind="ExternalInput")
    qr2d = nc.dram_tensor("q_r2", [2, C], BF16, kind="ExternalInput")
    yd = nc.dram_tensor("y", [BPC, C, N], F32, kind="ExternalOutput")

    with tile.TileContext(nc) as tc:
        with (
            tc.tile_pool(name="wpool", bufs=1) as wpool,
            tc.tile_pool(name="xpool", bufs=2) as xpool,
            tc.tile_pool(name="actpool", bufs=2) as actpool,
            tc.tile_pool(name="ppool", bufs=6) as ppool,
            tc.tile_pool(name="spool", bufs=2) as spool,
            tc.tile_pool(name="psum", bufs=2, space="PSUM") as psum,
            tc.tile_pool(name="psc", bufs=4, space="PSUM") as psc,
            tc.tile_pool(name="paug", bufs=2, space="PSUM") as paug,
            tc.tile_pool(name="dpool", bufs=4, space="DRAM") as dpool,
        ):
            # ---- persistent weights ----
            wq_sb = wpool.tile([P, CC, C], FP8)
            nc.scalar.dma_start(wq_sb[:], wq8d.ap().rearrange("(cc p) e -> p cc e", p=P))
            wck_sb = wpool.tile([P, DC, C], FP8)
            nc.scalar.dma_start(wck_sb[:], wck8d.ap().rearrange("(dc p) e -> p dc e", p=P))
            wcv_sb = wpool.tile([P, DC, C], FP8)
            nc.scalar.dma_start(wcv_sb[:], wcv8d.ap().rearrange("(dc p) e -> p dc e", p=P))
            wo_sb = wpool.tile([P, CC, C], FP8)
            nc.scalar.dma_start(wo_sb[:], wo8d.ap().rearrange("(ec p) c -> p ec c", p=P))
            qr2_sb = wpool.tile([2, C], BF16)
            nc.scalar.dma_start(qr2_sb[:], qr2d.ap())

            ones1_sb = wpool.tile([P, 1], BF16)   # stats lhsT (column sums)
            nc.vector.memset(ones1_sb[:], 1.0)
            onesr_sb = wpool.tile([1, P], BF16)    # bcast-matmul lhsT (rank-1)
            nc.vector.memset(onesr_sb[:], 1.0)
            eps2_sb = wpool.tile([1, 1], F32)      # eps * 2^20 (Rsqrt bias)
            nc.vector.memset(eps2_sb[:], EPS * 1048576.0)

            # Per-batch emission closures; emitted in a software-pipelined
            # order so PE filler (projection chains) sits between the
            # ACT-bound score-exp groups and their attn@v consumers.
            def make_batch(b):
                st = {}

                def loads():
                    # b0 bulk loads ride the sync queue; later batches use the
                    # gpsimd (SWDGE) queue so they don't delay the previous
                    # batch's latency-sensitive normalize DMAs on sync.
                    bulk = nc.sync.dma_start if b == 0 else nc.gpsimd.dma_start
                    st["x8"] = xpool.tile([P, CC, N], FP8, name=f"x8{b}", tag="x8")
                    st["xbf"] = xpool.tile([P, CC, N], BF16, name=f"xbf{b}", tag="xbf")
                    for cc in range(CC):
                        bulk(st["xbf"][:, cc, :],
                             xbfd.ap()[b][cc * P:(cc + 1) * P, :])
                    for cc in range(CC):
                        nc.scalar.dma_start(
                            st["x8"][:, cc, :],
                            x8d.ap()[b][cc * P:(cc + 1) * P, :])
                    st["ctx8"] = xpool.tile([P, DC, L], FP8, name=f"ctx8{b}", tag="ctx8")
                    for dc in range(DC):
                        bulk(st["ctx8"][:, dc, :],
                             ctx8d.ap()[b][dc * P:(dc + 1) * P, :])
                    st["mb"] = spool.tile([P, LC], F32, name=f"mb{b}", tag="mb")
                    nc.sync.dma_start(
                        st["mb"][:], mbd.ap()[b].rearrange("(lc p) -> p lc", p=P))
                    st["xsq"] = xpool.tile([P, CC, N], BF16, name=f"xsq{b}",
                                           tag="xsq", bufs=1)
                    for cc in range(CC):
                        nc.gpsimd.tensor_tensor(
                            st["xsq"][:, cc, :], st["xbf"][:, cc, :],
                            st["xbf"][:, cc, :], op=mybir.AluOpType.mult)
                    st["q"] = actpool.tile([P, CC, MC, 512], BF16, name=f"q{b}", tag="q")
                    st["k"] = actpool.tile([P, CC, L], BF16, name=f"k{b}", tag="k")
                    # v8: [d, lc-pair u, k-tile i, head, 96]; col 64 = 1/8 ones
                    # (denominator), cols 65:96 zero pad (DoubleRow stationary
                    # width must be a multiple of 32)
                    st["v8"] = actpool.tile([P, LC // 2, 2, NH, 96], FP8,
                                            name=f"v8{b}", tag="v8")
                    nc.vector.memset(st["v8"][:, :, :, :, HD + 1:], 0.0)
                    nc.vector.memset(st["v8"][:, :, :, :, HD:HD + 1], 0.125)
                    st["an8"] = actpool.tile([P, CC, MC, 512], FP8,
                                             name=f"an8{b}", tag="an8")
                    st["r2"] = {}
                    st["rbc"] = {}
                    st["den"] = {}
                    st["asb"] = {}
                    st["rcb"] = {}

                def stats(mc):
                    ms = slice(mc * 512, (mc + 1) * 512)
                    st1 = psum.tile([1, 512], F32, name=f"st1{b}{mc}", tag="ps")
                    for cc in range(CC):
                        nc.tensor.matmul(st1[:], ones1_sb[:], st["xbf"][:, cc, ms],
                                         start=(cc == 0), stop=(cc == CC - 1))
                    st2 = psum.tile([1, 512], F32, name=f"st2{b}{mc}", tag="ps")
                    for cc in range(CC):
                        nc.tensor.matmul(st2[:], ones1_sb[:], st["xsq"][:, cc, ms],
                                         start=(cc == 0), stop=(cc == CC - 1))
                    negmu = spool.tile([1, 512], BF16, name=f"negmu{b}{mc}", tag="negmu")
                    nc.vector.tensor_scalar_mul(negmu[:], st1[:], -1.0 / C)
                    musq = spool.tile([1, 512], F32, name=f"musq{b}{mc}", tag="musq")
                    nc.vector.tensor_tensor(musq[:], negmu[:], negmu[:],
                                            op=mybir.AluOpType.mult)
                    var = spool.tile([1, 512], F32, name=f"var{b}{mc}", tag="var")
                    nc.vector.scalar_tensor_tensor(
                        var[:], st2[:], 1.0 / C, musq[:],
                        op0=mybir.AluOpType.mult, op1=mybir.AluOpType.subtract)
                    # invr_k = sqrt((var+eps)*2^20) = 1024*invr  (fp32 for recip)
                    invr_k = spool.tile([1, 512], F32, name=f"invk{b}{mc}", tag="invk")
                    nc.scalar.activation(invr_k[:], var[:],
                                         mybir.ActivationFunctionType.Sqrt,
                                         bias=eps2_sb[:], scale=1048576.0)
                    rstd_f = spool.tile([1, 512], F32, name=f"rstf{b}{mc}", tag="rstf")
                    nc.vector.reciprocal_approx_fast(rstd_f[:], invr_k[:])
                    rstd = spool.tile([1, 512], BF16, name=f"rstd{b}{mc}", tag="rstd")
                    nc.vector.tensor_copy(rstd[:], rstd_f[:])
                    # invr/1024 = (var+eps) * (rstd/1024)
                    invr = spool.tile([1, 512], BF16, name=f"invr{b}{mc}", tag="invr")
                    nc.vector.scalar_tensor_tensor(
                        invr[:], var[:], EPS, rstd[:],
                        op0=mybir.AluOpType.add, op1=mybir.AluOpType.mult)
                    r2 = spool.tile([2, 512], BF16, name=f"r2_{b}{mc}", tag="r2")
                    nc.sync.dma_start(r2[0:1, :], negmu[:])
                    nc.sync.dma_start(r2[1:2, :], invr[:])
                    rbp = paug.tile([P, 512], F32, name=f"rbp{b}{mc}", tag="aug")
                    nc.tensor.matmul(rbp[:], onesr_sb[:], rstd[:],
                                     start=True, stop=True)
                    rbc = spool.tile([P, 512], BF16, name=f"rbc{b}{mc}", tag="rbc")
                    nc.vector.tensor_copy(rbc[:], rbp[:])
                    st["r2"][mc] = r2
                    st["rbc"][mc] = rbc

                def k_chain(ec):
                    es = slice(ec * P, (ec + 1) * P)
                    kp = psum.tile([P, 512], F32, name=f"kp{b}{ec}", tag="ps")
                    for u in range(DC // 2):
                        nc.tensor.matmul(kp[:], wck_sb[:, 2 * u:2 * u + 2, es],
                                         st["ctx8"][:, 2 * u:2 * u + 2, :],
                                         start=(u == 0), stop=(u == DC // 2 - 1),
                                         perf_mode=DR)
                    nc.vector.tensor_copy(st["k"][:, ec, :], kp[:])

                def v_chain(lc):
                    ls = slice(lc * P, (lc + 1) * P)
                    vp = psum.tile([P, 512], F32, name=f"vp{b}{lc}", tag="ps")
                    for u in range(DC // 2):
                        nc.tensor.matmul(vp[:], st["ctx8"][:, 2 * u:2 * u + 2, ls],
                                         wcv_sb[:, 2 * u:2 * u + 2, :],
                                         start=(u == 0), stop=(u == DC // 2 - 1),
                                         perf_mode=DR)
                    with nc.allow_low_precision(reason="fp8 attn values; error damped by residual"):
                        nc.vector.tensor_copy(
                            st["v8"][:, lc // 2, lc % 2, :, 0:HD],
                            vp[:].rearrange("p (h d) -> p h d", d=HD))

                def q_chain(ec, mc):
                    es = slice(ec * P, (ec + 1) * P)
                    ms = slice(mc * 512, (mc + 1) * 512)
                    qp = psum.tile([P, 512], F32, name=f"qp{b}{ec}{mc}", tag="ps")
                    for u in range(CC // 2):
                        nc.tensor.matmul(qp[:], wq_sb[:, 2 * u:2 * u + 2, es],
                                         st["x8"][:, 2 * u:2 * u + 2, ms],
                                         start=(u == 0), stop=False,
                                         perf_mode=DR)
                    nc.tensor.matmul(qp[:], qr2_sb[:, es], st["r2"][mc][:],
                                     start=False, stop=True)
                    nc.vector.tensor_tensor(st["q"][:, ec, mc, :], qp[:],
                                            st["rbc"][mc][:],
                                            op=mybir.AluOpType.mult)

                def sc_exp_group(mc, j):
                    if mc not in st["den"]:
                        st["den"][mc] = spool.tile([NH, 512], F32,
                                                   name=f"den{b}{mc}", tag="den")
                        st["asb"][mc] = {}
                    # probs for this head-pair: per hh a [P, 2(u: lc pair), 512]
                    # fp8 tile feeding the DoubleRow attn@v directly.
                    ps_h = []
                    for hh in range(2):
                        h = 2 * j + hh
                        po = hh * HD
                        p8 = ppool.tile([P, 2, 2, 512], FP8,
                                        name=f"p8{b}{mc}{j}{hh}", tag="p8")
                        ts = []
                        for lc in range(LC):
                            t = psc.tile([P, 512], F32,
                                         name=f"sc{b}{mc}{j}{hh}{lc}", tag="sc")
                            nc.tensor.matmul(
                                t[:],
                                st["k"][po:po + HD, j, lc * P:(lc + 1) * P],
                                st["q"][po:po + HD, j, mc, :],
                                start=True, stop=True)
                            ts.append(t)
                        for lc, t in enumerate(ts):
                            nc.scalar.activation(
                                p8[:, lc // 2, lc % 2, :], t[:],
                                mybir.ActivationFunctionType.Exp,
                                bias=st["mb"][:, lc:lc + 1])
                        ps_h.append(p8)
                    return ps_h

                def attnv_group(mc, j, ps_h):
                    for hh in range(2):
                        h = 2 * j + hh
                        aug = paug.tile([96, 512], F32,
                                        name=f"aug{b}{mc}{j}{hh}", tag="aug")
                        for u in range(2):
                            nc.tensor.matmul(aug[:], st["v8"][:, u, :, h, :],
                                             ps_h[hh][:, u, :, :],
                                             start=(u == 0), stop=(u == 1),
                                             perf_mode=DR)
                        asb = ppool.tile([HD + 1, 512], F32,
                                         name=f"asb{b}{mc}{j}{hh}", tag="asb",
                                         bufs=16)
                        nc.vector.tensor_copy(asb[:], aug[0:HD + 1, :])
                        nc.scalar.dma_start(st["den"][mc][h:h + 1, :],
                                            asb[HD:HD + 1, :])
                        st["asb"][mc][h] = asb

                def norm(mc):
                    rcpf = spool.tile([NH, 512], F32, name=f"rcpf{b}{mc}", tag="rcpf")
                    nc.vector.reciprocal_approx_fast(rcpf[:], st["den"][mc][:])
                    rcp = spool.tile([NH, 512], BF16, name=f"rcp{b}{mc}", tag="rcp")
                    nc.vector.tensor_copy(rcp[:], rcpf[:])
                    rcp_d = dpool.tile([NH, 512], BF16, name=f"rcpd{b}{mc}", tag="rcpd")
                    nc.scalar.dma_start(rcp_d[:], rcp[:])
                    for h in range(NH):
                        j, hh = h // 2, h % 2
                        rcb = spool.tile([HD, 512], BF16,
                                         name=f"rcb{b}{mc}{h}", tag="rcb",
                                         bufs=6)
                        nc.scalar.dma_start(
                            rcb[:], rcp_d[h:h + 1, :].to_broadcast((HD, 512)))
                        # split normalize between DVE and the idle gpsimd
                        eng = nc.vector if hh == 0 else nc.gpsimd
                        with nc.allow_low_precision(reason="fp8 attn operand; error damped by residual"):
                            eng.tensor_tensor(
                                st["an8"][hh * HD:(hh + 1) * HD, j, mc, :],
                                st["asb"][mc][h][0:HD, :], rcb[:],
                                op=mybir.AluOpType.mult)

                def outproj(cc, mc):
                    ms = slice(mc * 512, (mc + 1) * 512)
                    cs = slice(cc * P, (cc + 1) * P)
                    op = psum.tile([P, 512], F32, name=f"op{b}{cc}{mc}", tag="ps")
                    for u in range(CC // 2):
                        nc.tensor.matmul(op[:], wo_sb[:, 2 * u:2 * u + 2, cs],
                                         st["an8"][:, 2 * u:2 * u + 2, mc, :],
                                         start=(u == 0), stop=(u == CC // 2 - 1),
                                         perf_mode=DR)
                    y_sb = xpool.tile([P, 512], F32, name=f"y{b}{cc}{mc}",
                                      tag="y", bufs=4)
                    nc.vector.scalar_tensor_tensor(
                        y_sb[:], op[:], 1.0 / 4096.0, st["xbf"][:, cc, ms],
                        op0=mybir.AluOpType.mult, op1=mybir.AluOpType.add)
                    nc.sync.dma_start(
                        yd.ap()[b][cc * P:(cc + 1) * P, mc * 512:(mc + 1) * 512],
                        y_sb[:])

                return dict(loads=loads, stats=stats, k_chain=k_chain,
                            v_chain=v_chain, q_chain=q_chain,
                            sc_exp_group=sc_exp_group, attnv_group=attnv_group,
                            norm=norm, outproj=outproj)

            # ---- software-pipelined emission (cross-batch modulo schedule) ----
            # PE fillers sit between ACT-bound score/exp groups and their
            # attn@v consumers; fillers are chosen to be independent of the
            # preceding normalize latency.
            E = [make_batch(b) for b in range(BPC)]

            def attn_pass(eb, mc, fillers, post_first=None):
                # attn@v trails the score/exp groups by one j so its P tiles
                # (ACT exps) are complete; fillers keep PE fed in between.
                prev = None
                for j in range(NH // 2):
                    ps_h = eb["sc_exp_group"](mc, j)
                    fillers[j]()
                    if prev is not None:
                        eb["attnv_group"](mc, j - 1, prev)
                        if j == 1 and post_first is not None:
                            post_first()
                    prev = ps_h
                eb["attnv_group"](mc, NH // 2 - 1, prev)

            def nop():
                pass

            E[0]["loads"]()
            E[0]["stats"](0)
            E[0]["stats"](1)
            for lc in range(LC):
                E[0]["v_chain"](lc)
            E[0]["k_chain"](0)
            E[0]["q_chain"](0, 0)
            E[0]["q_chain"](0, 1)

            def kq(eb, j):
                def f():
                    eb["k_chain"](j)
                    eb["q_chain"](j, 0)
                    eb["q_chain"](j, 1)
                return f

            attn_pass(E[0], 0, [kq(E[0], 1), kq(E[0], 2), kq(E[0], 3),
                                E[1]["loads"]])
            attn_pass(E[0], 1,
                      [lambda: (E[1]["stats"](0), E[1]["stats"](1)),
                       lambda: (E[1]["v_chain"](0), E[1]["v_chain"](1)),
                       lambda: (E[1]["v_chain"](2), E[1]["v_chain"](3)),
                       kq(E[1], 0)],
                      post_first=lambda: E[0]["norm"](0))
            attn_pass(E[1], 0,
                      [lambda: (E[0]["outproj"](0, 0), E[1]["k_chain"](1),
                                E[1]["q_chain"](1, 0), E[1]["q_chain"](1, 1)),
                       lambda: (E[0]["outproj"](1, 0), E[1]["k_chain"](2),
                                E[1]["q_chain"](2, 0), E[1]["q_chain"](2, 1)),
                       lambda: (E[0]["outproj"](2, 0), E[1]["k_chain"](3),
                                E[1]["q_chain"](3, 0), E[1]["q_chain"](3, 1)),
                       lambda: E[0]["outproj"](3, 0)],
                      post_first=lambda: E[0]["norm"](1))
            attn_pass(E[1], 1,
                      [lambda: E[0]["outproj"](0, 1),
                       lambda: E[0]["outproj"](1, 1),
                       lambda: (E[0]["outproj"](2, 1), E[1]["outproj"](0, 0)),
                       lambda: (E[0]["outproj"](3, 1), E[1]["outproj"](1, 0))],
                      post_first=lambda: E[1]["norm"](0))
            E[1]["outproj"](2, 0)
            E[1]["outproj"](3, 0)
            E[1]["norm"](1)
            for cc in range(CC):
                E[1]["outproj"](cc, 1)
    nc.compile()
    return nc


def _get_nc():
    global _NC_CACHE
    if _NC_CACHE is None:
        _NC_CACHE = _build()
    return _NC_CACHE


def kernel(x, context, context_mask, ln_w, ln_b, Wq, Wk, Wv, Wo, Wctx):
    x = np.asarray(x, np.float32)
    context = np.asarray(context, np.float32)
    context_mask = np.asarray(context_mask)
    ln_w = np.asarray(ln_w, np.float32)
    ln_b = np.asarray(ln_b, np.float32)
    Wq = np.asarray(Wq, np.float32)
    Wk = np.asarray(Wk, np.float32)
    Wv = np.asarray(Wv, np.float32)
    Wo = np.asarray(Wo, np.float32)
    Wctx = np.asarray(Wctx, np.float32)

    scale = HD ** -0.5
    wq_f = Wq * (ln_w[None, :] * scale)          # [E, C] ln scale + attn scale
    wq8 = np.ascontiguousarray(wq_f.T * 64.0).astype(F8)
    q_r2 = np.stack([64.0 * wq_f.sum(1),
                     65536.0 * ((Wq * scale) @ ln_b)]).astype(BF)   # [2, E]
    wck8 = np.ascontiguousarray((Wk @ Wctx).T * 16.0).astype(F8)    # [768, 512]
    wcv8 = np.ascontiguousarray((Wv @ Wctx).T * 16.0).astype(F8)
    wo8 = np.ascontiguousarray(Wo.T * 32.0).astype(F8)

    xr = x.reshape(NCORES, BPC, C, N)
    x8 = xr.astype(F8)
    xbf = xr.astype(BF)
    ctx8 = np.ascontiguousarray(
        context.transpose(0, 2, 1)).astype(F8).reshape(NCORES, BPC, CTX_DIM, L)
    mb = ((~context_mask).astype(np.float32) * MASK_NEG).reshape(NCORES, BPC, L)

    in_maps = [
        {"x8": np.ascontiguousarray(x8[c]), "xbf": np.ascontiguousarray(xbf[c]),
         "ctx8": np.ascontiguousarray(ctx8[c]),
         "mb": np.ascontiguousarray(mb[c]), "wq8": wq8, "wck8": wck8,
         "wcv8": wcv8, "wo8": wo8, "q_r2": q_r2}
        for c in range(NCORES)
    ]
    res = run_bass_kernel_spmd(_get_nc(), in_maps, core_ids=list(range(NCORES)))
    y = np.stack([r["y"] for r in res.results])          # [8, 2, C, N]
    return y.reshape(B, C, H, W)
